# revision 12
# baseline (speedup 1.0000x reference)
"""Trainium2 Bass kernel for nn_DensityLoss (raw Block mode, SPMD x8, replicated).

Math
----
reference(centers, features, labels) depends only on centers [C=4096, D=256]
(features unused; labels only via N=len(labels)=262144, a constant):

    sq_i  = ||c_i||^2;  m = sum_i c_i;  S = sum sq;  proj_i = c_i . m
    n_i   = C*sq_i + S - 2*proj_i          (center_dist_i = n_i/(C-1); diag==0)
    sum n   = 2*C*S - 2*m.m
    sum n^2 = C^2 q + 3C S^2 + 4 m'Sigma m - 4C (w.m) - 4S (m.m)
        q = sum sq^2, w = sum sq_i c_i, Sigma = X'X
        (sum proj = m.m, sum proj^2 = m'Sigma m)
    result = (sum n) (C-1)^2 / (C * N * (sum n^2 - (sum n)^2/C))

Everything comes from the Gram of Xhat=[X | 1 | sq/256] ([4096, 258]),
accumulated in PSUM over 32 row-tiles of 128:
    psA = G[0:128, 0:258]   (Sigma row-block 0, m0 col 256, w0 col 257)
    psB = G[128:256,128:258](Sigma block 11 + B01 implied by symmetry, m1, w1)
    psC = G[256:258,256:258]([[C, S'], [S', q']]),  S'=S/256, q'=q/65536
sq/256 per row: DVE bn_stats (mean^2+var) on even tiles, ACT Square(x/16)
with accum_out on odd tiles. m'Sigma m via three [128,128]x[128,1] matvecs.
Scalar tail on one partition. Centers replicated to all 8 cores (an 8-core
AllReduce has a ~10us floor - more than this whole kernel).
"""

import numpy as np

C, D = 4096, 256
N_LABELS = 262144
P = 128
NT = C // P            # 32 row tiles
W = D + 2              # 258: [X | ones | sq/256]
CH = 4                 # tiles per DMA chunk
NCHUNK = NT // CH      # 8 chunks
N_CORES = 8
DMA_INC = 16         # one +16 per chunk dma_start (per-chunk semaphore)

_CACHE = {}


def _build_nc(repeat=1):
    import concourse.bass as bass
    from concourse import mybir

    f32 = mybir.dt.float32
    Alu = mybir.AluOpType
    Act = mybir.ActivationFunctionType

    nc = bass.Bass()
    x_ext = nc.declare_dram_parameter("centers", [C, D], f32, isOutput=False)
    out_ext = nc.declare_dram_parameter("out", [1, 1], f32, isOutput=True)

    xv = x_ext[:, :].rearrange("(t p) d -> p t d", p=P)   # [128, 32, 256] view

    from contextlib import ExitStack

    with ExitStack() as ctx:
        en = ctx.enter_context
        xh = en(nc.sbuf_tensor([P, NT, W], f32))
        st6 = en(nc.sbuf_tensor([P, NT // 2, 6], f32))
        mv2 = en(nc.sbuf_tensor([P, NT // 2, 2], f32))
        scr_a = en(nc.sbuf_tensor([P, NT // 2, D], f32))
        zc = en(nc.sbuf_tensor([P, 1], f32))
        ones_col = en(nc.sbuf_tensor([P, 1], f32))
        ones2 = en(nc.sbuf_tensor([2, 1], f32))
        Ga = en(nc.sbuf_tensor([P, W], f32))
        Gb = en(nc.sbuf_tensor([P, W - P], f32))
        Gc = en(nc.sbuf_tensor([2, 2], f32))
        e = en(nc.sbuf_tensor([P, 7], f32))
        sc = en(nc.sbuf_tensor([1, 32], f32))
        res = en(nc.sbuf_tensor([1, 1], f32))
        psA = en(nc.psum_tensor([P, W], f32))
        psB = en(nc.psum_tensor([P, W - P], f32))
        psC = en(nc.psum_tensor([2, 2], f32))
        pv0 = en(nc.psum_tensor([P, 1], f32))
        pt1 = en(nc.psum_tensor([P, 1], f32))
        pv1 = en(nc.psum_tensor([P, 1], f32))
        psS = en(nc.psum_tensor([1, 9], f32))
        s_dma = [en(nc.semaphore(f"s_dma{i}")) for i in range(NCHUNK)]
        s_pre = en(nc.semaphore("s_pre"))
        s_sqv = en(nc.semaphore("s_sqv"))
        s_sqa = en(nc.semaphore("s_sqa"))
        s_mm = en(nc.semaphore("s_mm"))
        s_cpa = en(nc.semaphore("s_cpa"))
        s_cpb = en(nc.semaphore("s_cpb"))
        s_mv = en(nc.semaphore("s_mv"))
        s_e = en(nc.semaphore("s_e"))
        s_sum = en(nc.semaphore("s_sum"))
        s_res = en(nc.semaphore("s_res"))
        s_out = en(nc.semaphore("s_out"))
        block = en(nc.Block())
        m0 = Ga[:, D:D + 1]
        w0 = Ga[:, D + 1:D + 2]
        m1 = Gb[:, D - P:D - P + 1]
        w1 = Gb[:, D - P + 1:D - P + 2]

        @block.sync
        def _(sync):
            for _r in range(repeat):
                for ci in range(NCHUNK):
                    sync.dma_start(
                        out=xh[:, ci * CH:(ci + 1) * CH, 0:D],
                        in_=xv[:, ci * CH:(ci + 1) * CH, :],
                    ).then_inc(s_dma[ci], 16)
            sync.wait_ge(s_res, 1)
            sync.dma_start(out=out_ext[:, :], in_=res[:, :]).then_inc(s_out, 16)
            sync.wait_ge(s_out, 16)

        @block.vector
        def _(vector):
            # preamble constants (cols disjoint from the DMA'd cols 0:256)
            vector.memset(xh[:, :, D:D + 1], 1.0)
            vector.memset(zc[:, :], 0.0)
            vector.memset(ones_col[:, :], 1.0)
            nc.vector.memset(ones2[:, :], 1.0).then_inc(s_pre, 1)
            # sq/256 for even tiles: bn_stats -> mean^2 + var.  Processed in
            # groups of 4 tiles with phase-wise drains (the DVE pipeline does
            # not guarantee RAW ordering within the engine; per-tile slots in
            # st6/mv2 make ops within a phase independent).
            GR = 4
            for _r in range(repeat):
                for g in range(0, NT // 2, GR):
                    tiles = [2 * (g + j) for j in range(GR)]
                    for j, t in enumerate(tiles):
                        vector.wait_ge(s_dma[t // CH], DMA_INC * (_r + 1))
                        nc.vector.bn_stats(out=st6[:, g + j, :], in_=xh[:, t, 0:D])
                    vector.drain()
                    for j in range(GR):
                        nc.vector.bn_aggr(out=mv2[:, g + j, :], in_=st6[:, g + j, :])
                    vector.drain()
                    for j, t in enumerate(tiles):
                        nc.vector.tensor_scalar(
                            xh[:, t, D + 1:D + 2],
                            mv2[:, g + j, 0:1], mv2[:, g + j, 0:1], mv2[:, g + j, 1:2],
                            op0=Alu.mult, op1=Alu.add,
                        ).then_inc(s_sqv, 1)
            # Gram -> SBUF (psB/psC here, psA on ACT in parallel)
            vector.wait_ge(s_mm, 1)
            nc.vector.tensor_copy(Gb[:, :], psB[:, :])
            nc.vector.tensor_copy(Gc[:, :], psC[:, :]).then_inc(s_cpb, 1)
            # dot-product columns
            vector.wait_ge(s_cpa, 1)
            vector.wait_ge(s_mv, 1)
            vector.drain()
            nc.vector.tensor_mul(e[:, 0:1], pv0[:, :], m0)
            nc.vector.tensor_mul(e[:, 1:2], pv1[:, :], m1)
            nc.vector.tensor_mul(e[:, 2:3], pt1[:, :], m1)
            nc.vector.tensor_mul(e[:, 3:4], m0, m0)
            nc.vector.tensor_mul(e[:, 4:5], m1, m1)
            nc.vector.tensor_mul(e[:, 5:6], w0, m0)
            nc.vector.tensor_mul(e[:, 6:7], w1, m1).then_inc(s_e, 1)
            # scalar tail on partition 0 (drain between dependent ops)
            vector.wait_ge(s_sum, 1)

            def TS(*a, **kw):
                vector.drain()
                return nc.vector.tensor_scalar(*a, **kw)

            def TT(*a, **kw):
                vector.drain()
                return nc.vector.tensor_tensor(*a, **kw)

            def s(i):
                return sc[:, i:i + 1]

            Cf = float(C)
            # sc: 0 v0m0 | 1 v1m1 | 2 t1m1 | 3 m0m0 | 4 m1m1 | 5 w0m0' | 6 w1m1'
            #     7 C+S' | 8 S'+q'
            nc.vector.tensor_copy(sc[:, 0:7], psS[0:1, 0:7])
            nc.vector.tensor_copy(sc[:, 7:9], psS[0:1, 7:9])
            TS(s(9), s(7), -Cf, None, op0=Alu.add)                         # S'
            TT(s(10), s(8), s(9), op=Alu.subtract)                         # q'
            TT(s(11), s(3), s(4), op=Alu.add)                              # mm
            TT(s(12), s(0), s(1), op=Alu.add)
            TS(s(13), s(2), 2.0, s(12), op0=Alu.mult, op1=Alu.add)         # mSm
            TT(s(14), s(5), s(6), op=Alu.add)                              # w'm
            TS(s(15), s(9), 512.0 * Cf, None, op0=Alu.mult)                # 2CS
            TS(s(16), s(11), -2.0, s(15), op0=Alu.mult, op1=Alu.add)       # Sn
            TT(s(17), s(9), s(9), op=Alu.mult)                             # S'^2
            TS(s(18), s(17), 3.0 * Cf * 65536.0, None, op0=Alu.mult)       # 3CS^2
            TS(s(19), s(10), Cf * Cf * 65536.0, s(18), op0=Alu.mult, op1=Alu.add)
            TS(s(20), s(13), 4.0, s(19), op0=Alu.mult, op1=Alu.add)        # +4mSm
            TS(s(21), s(14), -1024.0 * Cf, s(20), op0=Alu.mult, op1=Alu.add)
            TT(s(22), s(9), s(11), op=Alu.mult)                            # S'*mm
            TS(s(23), s(22), -1024.0, s(21), op0=Alu.mult, op1=Alu.add)    # Sn2
            TT(s(24), s(16), s(16), op=Alu.mult)                           # Sn^2
            TS(s(25), s(24), -1.0 / Cf, s(23), op0=Alu.mult, op1=Alu.add)  # d
            TS(s(26), s(25), 2.0 ** -20, None, op0=Alu.mult)               # d*2^-20
            vector.drain()
            nc.vector.reciprocal(s(27), s(26))
            TT(s(28), s(16), s(27), op=Alu.mult)
            k = (Cf - 1.0) ** 2 / (Cf * float(N_LABELS)) * (2.0 ** -20)
            TS(res[:, :], s(28), k, None, op0=Alu.mult).then_inc(s_res, 1)

        @block.scalar
        def _(scalar):
            scalar.wait_ge(s_pre, 1)
            # sq/256 for odd tiles: accum(Square(x/16))
            for _r in range(repeat):
                for j, t in enumerate(range(1, NT, 2)):
                    scalar.wait_ge(s_dma[t // CH], DMA_INC * (_r + 1))
                    nc.scalar.activation(
                        out=scr_a[:, j, :], in_=xh[:, t, 0:D], func=Act.Square,
                        bias=zc[:, :], scale=0.0625,
                        accum_out=xh[:, t, D + 1:D + 2],
                    ).then_inc(s_sqa, 1)
            scalar.wait_ge(s_mm, 1)
            nc.scalar.copy(Ga[:, :], psA[:, :]).then_inc(s_cpa, 1)

        @block.tensor
        def _(tensor):
            tensor.wait_ge(s_pre, 1)
            for _r in range(repeat):
                for t in range(NT):
                    tensor.wait_ge(s_sqv if t % 2 == 0 else s_sqa,
                                   _r * (NT // 2) + t // 2 + 1)
                    first = (_r == 0 and t == 0)
                    last = (_r == repeat - 1 and t == NT - 1)
                    nc.tensor.matmul(psA[:, :], xh[:, t, 0:P], xh[:, t, :],
                                     start=first, stop=last)
                    nc.tensor.matmul(psB[:, :], xh[:, t, P:D], xh[:, t, P:W],
                                     start=first, stop=last)
                    mm = nc.tensor.matmul(psC[:, :], xh[:, t, D:W], xh[:, t, D:W],
                                          start=first, stop=last)
                    if last:
                        mm.then_inc(s_mm, 1)
            tensor.wait_ge(s_cpa, 1)
            tensor.wait_ge(s_cpb, 1)
            nc.tensor.matmul(pv0[:, :], Ga[:, 0:P], m0, start=True, stop=True)
            nc.tensor.matmul(pt1[:, :], Ga[:, P:D], m0, start=True, stop=True)
            nc.tensor.matmul(pv1[:, :], Gb[:, 0:P], m1,
                             start=True, stop=True).then_inc(s_mv, 1)
            tensor.wait_ge(s_e, 1)
            nc.tensor.matmul(psS[:, 0:7], ones_col[:, :], e[:, 0:7],
                             start=True, stop=True)
            nc.tensor.matmul(psS[:, 7:9], ones2[:, :], Gc[:, :],
                             start=True, stop=True).then_inc(s_sum, 1)

    return nc


def _get_nc(repeat=1):
    key = ("nc", repeat)
    if key not in _CACHE:
        _CACHE[key] = _build_nc(repeat)
    return _CACHE[key]


def run(centers: np.ndarray, trace: bool = False):
    """Run the SPMD kernel on cores 0-7; returns (scalar ndarray, results)."""
    from concourse.bass_utils import run_bass_kernel_spmd

    nc = _get_nc()
    x = np.ascontiguousarray(np.asarray(centers, dtype=np.float32))
    in_maps = [{"centers": x} for _ in range(N_CORES)]
    r = run_bass_kernel_spmd(nc, in_maps, core_ids=list(range(N_CORES)),
                             trace=trace)
    out = np.asarray(r.results[0]["out"], dtype=np.float32).reshape(())
    return out, r


def kernel(centers, features=None, labels=None, **_):
    out, _r = run(centers)
    return out
